# revision 9
# baseline (speedup 1.0000x reference)
"""BitAttention (BitNet-style fake-quant attention) on 8 Trainium2 NeuronCores.

Sharding:
  - activation quant (int8 codes) is token-sharded (512 tokens/core); codes are
    AllGather'ed as bf16 (codes in [-127,127] are bf16-exact).
  - QKV projections + attention are head-sharded (2 heads/core): each core
    computes q/k/v for its heads over all 4096 tokens as exact integer
    matmuls (bf16 activation codes x ternary bf16 weight codes, fp32 PSUM).
  - weight quant scales (mean|w|) need a global mean -> [1,4] AllReduce.
  - attention computes scores^T per (b,h); softmax skips max-subtraction
    (|s|_max ~ 1.5 for this problem); exp on ScalarE; AV as E^T-stationary
    matmul with moving operand [v | k | 1]: the k block yields E@K for the
    entropy term, the ones column the softmax denominator.
  - attention outputs AllToAll back to token shards for the O projection
    (row quant needs whole rows).
  - entropy partials are summed on host (output unsharding).
"""

import os
import sys

for _p in ("/opt/trn_rl_repo",):
    if _p not in sys.path and os.path.isdir(_p):
        sys.path.append(_p)

import numpy as np

import concourse.bacc as bacc
import concourse.bass as bass
import concourse.mybir as mybir
import concourse.tile as tile
from concourse.bass_utils import run_bass_kernel_spmd
from concourse.masks import make_identity

F32 = mybir.dt.float32
BF16 = mybir.dt.bfloat16
AF = mybir.ActivationFunctionType
OP = mybir.AluOpType
AX = mybir.AxisListType

NCORES = 8
B, T, D, H, DH = 2, 2048, 1024, 16, 64
N = B * T                  # 4096 tokens
TOK = N // NCORES          # 512 tokens per core shard
FPC = 2 * DH               # 128 feature cols per core (2 heads)
NTT = N // 128             # 32 token tiles
SCALE = 0.125
MAGIC = float(np.float32(1.5 * 2 ** 23))
INV127 = float(np.float32(1.0) / np.float32(127.0))
WMEAN = float(np.float32(1.0 / (D * D)))
AVW = 132                  # av psum slot stride (129 used), 3 per bank


def _slot(qt):
    return (qt // 3) * 512 + (qt % 3) * AVW


def build_program():
    nc = bacc.Bacc("TRN2", target_bir_lowering=False, debug=False,
                   num_devices=NCORES)

    xs = nc.dram_tensor("xs", [TOK, D], F32, kind="ExternalInput")
    wt_qkv = nc.dram_tensor("wt_qkv", [D, 3 * FPC], F32, kind="ExternalInput")
    wo_t = nc.dram_tensor("wo_t", [D, D], F32, kind="ExternalInput")
    bias_qkv = nc.dram_tensor("bias_qkv", [128, 3 * FPC], F32, kind="ExternalInput")
    bo_bc = nc.dram_tensor("bo_bc", [128, D], F32, kind="ExternalInput")
    out_d = nc.dram_tensor("out", [TOK, D], F32, kind="ExternalOutput")
    ent_d = nc.dram_tensor("entp", [1, 1], F32, kind="ExternalOutput")

    agin = nc.dram_tensor("agin", [1028, 512], BF16)
    agout = nc.dram_tensor("agout", [NCORES, 1028, 512], BF16, addr_space="Shared")
    arin = nc.dram_tensor("arin", [1, 4], F32)
    arout = nc.dram_tensor("arout", [1, 4], F32, addr_space="Shared")
    a2a_in = nc.dram_tensor("a2a_in", [NCORES, TOK, FPC], F32)
    a2a_out = nc.dram_tensor("a2a_out", [NCORES, TOK, FPC], F32)
    dbg = os.environ.get("BITATTN_DEBUG")
    if dbg:
        dbg_qnat = nc.dram_tensor("dbg_qnat", [128, NTT * FPC], BF16)
        dbg_qT = nc.dram_tensor("dbg_qT", [128, N], BF16)
        dbg_kT = nc.dram_tensor("dbg_kT", [128, N], BF16)
        dbg_vk1 = nc.dram_tensor("dbg_vk1", [128, NTT * 2 * AVW], BF16)
        dbg_dvec = nc.dram_tensor("dbg_dvec", [128, 64], F32)
        dbg_edot = nc.dram_tensor("dbg_edot", [128, 64], F32)
        dbg_avm = nc.dram_tensor("dbg_avm", [4, 128, D], F32)
        dbg_oq = nc.dram_tensor("dbg_oq", [4, 128, D], BF16)
        dbg_alo = nc.dram_tensor("dbg_alo", [4, 128, 1], F32)
        dbg_gb = nc.dram_tensor("dbg_gb", [128, 8], F32)

    rg = [list(range(NCORES))]

    with tile.TileContext(nc) as tc:
        with (
            tc.tile_pool(name="const", bufs=1) as cpool,
            tc.tile_pool(name="work", bufs=2) as wpool,
            tc.tile_pool(name="small", bufs=2) as spool,
            tc.tile_pool(name="dmal", bufs=4) as dpool,
        ):
            # ---------- long-lived SBUF ----------
            ident = cpool.tile([128, 128], BF16, tag="ident")
            make_identity(nc, ident[:])
            ones_r = cpool.tile([1, 128], F32, tag="ones_r")
            nc.vector.memset(ones_r[:], 1.0)
            ones_c = cpool.tile([128, 1], F32, tag="ones_c")
            nc.vector.memset(ones_c[:], 1.0)

            bias_sb = cpool.tile([128, 3 * FPC], F32, tag="bias_sb")
            nc.sync.dma_start(bias_sb[:], bias_qkv.ap())
            bo_sb = cpool.tile([128, D], F32, tag="bo_sb")
            nc.sync.dma_start(bo_sb[:], bo_bc.ap())

            wt_all = cpool.tile([128, 8 * 3 * FPC], F32, tag="wt_all")
            for ic in range(8):
                nc.sync.dma_start(wt_all[:, ic * 384:(ic + 1) * 384],
                                  wt_qkv.ap()[ic * 128:(ic + 1) * 128, :])

            wc_sb = cpool.tile([128, 8 * 3 * FPC], BF16, tag="wc_sb")
            woc_sb = cpool.tile([128, 8 * D], BF16, tag="woc_sb")
            gb = cpool.tile([128, 8], F32, tag="gb")       # 0-3 gam(q,k,v,o), 4-7 recip
            al_all = cpool.tile([128, NTT], F32, tag="al_all")
            agq = cpool.tile([128, 3 * NTT], F32, tag="agq")
            q_nat = cpool.tile([128, NTT * FPC + 64], BF16, tag="q_nat")
            q_T = cpool.tile([128, N], BF16, tag="q_T")
            k_T = cpool.tile([128, N], BF16, tag="k_T")
            vk1 = cpool.tile([128, NTT * 2 * AVW], BF16, tag="vk1")
            nc.vector.memset(vk1[:], 1.0)
            av_sb = cpool.tile([128, 4 * 1024], F32, tag="av_sb")
            dvec_all = cpool.tile([128, 64], F32, tag="dvec_all")
            edot_all = cpool.tile([128, 64], F32, tag="edot_all")
            ent_scr = cpool.tile([128, 16 * DH], F32, tag="ent_scr")
            rexp = cpool.tile([128, 16 * DH], F32, tag="rexp")

            # ================= phase A: |w| partials -> AllReduce ============
            with tc.tile_pool(name="psa", bufs=1, space="PSUM") as psa:
                accw = spool.tile([128, 3, 9], F32, tag="accw")
                for ic in range(8):
                    nc.vector.tensor_reduce(
                        accw[:, :, ic:ic + 1],
                        wt_all[:, ic * 384:(ic + 1) * 384].rearrange(
                            "p (g f) -> p g f", g=3),
                        AX.X, OP.add, apply_absolute_value=True)
                accw2 = spool.tile([128, 3], F32, tag="accw2")
                nc.vector.tensor_reduce(accw2[:], accw[:, :, 0:8], AX.X, OP.add)

                accwo = spool.tile([128, 1, 9], F32, tag="accwo")
                for ic in range(8):
                    wot = wpool.tile([128, D], F32, tag="wot")
                    nc.sync.dma_start(wot[:],
                                      wo_t.ap()[ic * 128:(ic + 1) * 128, :])
                    nc.vector.tensor_reduce(accwo[:, :, ic:ic + 1],
                                            wot[:].unsqueeze(1), AX.X, OP.add,
                                            apply_absolute_value=True)
                accwo2 = spool.tile([128, 1], F32, tag="accwo2")
                nc.vector.tensor_reduce(accwo2[:], accwo[:, :, 0:8], AX.X, OP.add)
                # every core reduced the FULL wo; the AllReduce would count it
                # 8x -> pre-scale by 1/8 (exact: 8 identical fp32 summands)
                nc.vector.tensor_scalar(accwo2[:], accwo2[:], 0.125, None,
                                        OP.mult)

                psum_w = psa.tile([1, 4], F32, tag="psum_w")
                nc.tensor.matmul(psum_w[:, 0:3], ones_c[:], accw2[:],
                                 start=True, stop=True)
                nc.tensor.matmul(psum_w[:, 3:4], ones_c[:], accwo2[:],
                                 start=True, stop=True)
                wsum_sb = spool.tile([1, 4], F32, tag="wsum_sb")
                nc.vector.tensor_copy(wsum_sb[:], psum_w[:])
                nc.sync.dma_start(arin.ap(), wsum_sb[:])
                nc.gpsimd.collective_compute("AllReduce", OP.add,
                                             replica_groups=rg,
                                             ins=[arin.ap()], outs=[arout.ap()])

                # ============ phase B: x quant -> AllGather ==================
                with tc.tile_pool(name="psb", bufs=2, space="PSUM") as psb:
                    for i in range(TOK // 128):
                        x_sb = wpool.tile([128, D], F32, tag="x_sb")
                        nc.sync.dma_start(x_sb[:],
                                          xs.ap()[i * 128:(i + 1) * 128, :])
                        rmax = spool.tile([128, 1], F32, tag="rmax")
                        nc.vector.tensor_reduce(rmax[:], x_sb[:], AX.X, OP.max,
                                                apply_absolute_value=True)
                        rmax2 = spool.tile([128, 1], F32, tag="rmax2")
                        nc.vector.tensor_scalar(rmax2[:], rmax[:], 1e-5, None,
                                                OP.max)
                        srec = spool.tile([128, 1], F32, tag="srec")
                        nc.vector.reciprocal(srec[:], rmax2[:])
                        sq = spool.tile([128, 1], F32, tag="sq")
                        nc.vector.tensor_scalar(sq[:], srec[:], 127.0, None,
                                                OP.mult)
                        alpha = spool.tile([128, 1], F32, tag="alpha")
                        nc.vector.tensor_scalar(alpha[:], rmax2[:], INV127, None,
                                                OP.mult)

                        t1 = wpool.tile([128, D], F32, tag="t1")
                        nc.vector.tensor_scalar(t1[:], x_sb[:], sq[:], MAGIC,
                                                OP.mult, OP.add)
                        xq = wpool.tile([128, D], BF16, tag="xq")
                        nc.vector.tensor_scalar(xq[:], t1[:], MAGIC, None,
                                                OP.subtract)

                        xqt = wpool.tile([128, D], BF16, tag="xqt")
                        for ib in range(8):
                            pst = psb.tile([128, 128], BF16, tag="pst")
                            nc.tensor.transpose(
                                pst[:], xq[:, ib * 128:(ib + 1) * 128], ident[:])
                            nc.vector.tensor_copy(
                                xqt[:, ib * 128:(ib + 1) * 128], pst[:])
                        nc.sync.dma_start(
                            agin.ap()[0:1024, i * 128:(i + 1) * 128].rearrange(
                                "(ib p) t -> p ib t", p=128),
                            xqt[:].rearrange("p (ib t) -> p ib t", ib=8))
                        nc.sync.dma_start(
                            agin.ap().bitcast(F32)[1024 + i:1025 + i,
                                                   0:128].transpose([1, 0]),
                            alpha[:])

                    nc.gpsimd.collective_compute("AllGather", OP.bypass,
                                                 replica_groups=rg,
                                                 ins=[agin.ap()],
                                                 outs=[agout.ap()])

                # gammas + broadcast (waits on AllReduce result)
                gam4 = spool.tile([1, 4], F32, tag="gam4")
                nc.sync.dma_start(gam4[:], arout.ap())
                gam8 = spool.tile([1, 8], F32, tag="gam8")
                nc.vector.tensor_scalar(gam8[:, 0:4], gam4[:], WMEAN, 1e-5,
                                        OP.mult, OP.max)
                nc.vector.reciprocal(gam8[:, 4:8], gam8[:, 0:4])
                psum_g = psa.tile([128, 8], F32, tag="psum_g")
                nc.tensor.matmul(psum_g[:], ones_r[:], gam8[:],
                                 start=True, stop=True)
                nc.vector.tensor_copy(gb[:], psum_g[:])

            # ================ phase C: quantize weights ======================
            for ic in range(8):
                for g in range(3):
                    wsl = wt_all[:, ic * 384 + g * 128: ic * 384 + (g + 1) * 128]
                    tq = wpool.tile([128, 128], F32, tag="tq")
                    nc.vector.tensor_scalar(tq[:], wsl, gb[:, 4 + g:5 + g],
                                            MAGIC, OP.mult, OP.add)
                    tq2 = wpool.tile([128, 128], F32, tag="tq2")
                    nc.vector.tensor_scalar(tq2[:], tq[:], MAGIC + 1.0,
                                            MAGIC - 1.0, OP.min, OP.max)
                    nc.vector.tensor_scalar(
                        wc_sb[:, ic * 384 + g * 128: ic * 384 + (g + 1) * 128],
                        tq2[:], MAGIC, None, OP.subtract)
            for ic in range(8):
                wot = wpool.tile([128, D], F32, tag="wot")
                nc.sync.dma_start(wot[:], wo_t.ap()[ic * 128:(ic + 1) * 128, :])
                two = wpool.tile([128, D], F32, tag="two")
                nc.vector.tensor_scalar(two[:], wot[:], gb[:, 7:8], MAGIC,
                                        OP.mult, OP.add)
                two2 = wpool.tile([128, D], F32, tag="two2")
                nc.vector.tensor_scalar(two2[:], two[:], MAGIC + 1.0,
                                        MAGIC - 1.0, OP.min, OP.max)
                nc.vector.tensor_scalar(woc_sb[:, ic * D:(ic + 1) * D], two2[:],
                                        MAGIC, None, OP.subtract)

            # ================ phase D: QKV projection ========================
            with tc.tile_pool(name="psd", bufs=2, space="PSUM") as psd:
                for c in range(NCORES):
                    nc.sync.dma_start(
                        al_all[:, c * 4:(c + 1) * 4],
                        agout.ap().bitcast(F32)[c, 1024:1028,
                                                0:128].transpose([1, 0]))
                for g in range(3):
                    nc.vector.tensor_scalar(agq[:, g * NTT:(g + 1) * NTT],
                                            al_all[:], gb[:, g:g + 1], None,
                                            OP.mult)

                for tt in range(NTT):
                    c, l = tt // 4, tt % 4
                    psq = psd.tile([128, 3 * FPC], F32, tag="psq")
                    for ic in range(8):
                        lhs = dpool.tile([128, 128], BF16, tag="lhs")
                        nc.sync.dma_start(
                            lhs[:], agout.ap()[c, ic * 128:(ic + 1) * 128,
                                               l * 128:(l + 1) * 128])
                        nc.tensor.matmul(psq[:], lhs[:],
                                         wc_sb[:, ic * 384:(ic + 1) * 384],
                                         start=(ic == 0), stop=(ic == 7))
                    nc.vector.scalar_tensor_tensor(
                        q_nat[:, tt * FPC:(tt + 1) * FPC], psq[:, 0:128],
                        agq[:, tt:tt + 1], bias_sb[:, 0:128], OP.mult, OP.add)
                    for hr in range(2):
                        base = (tt * 2 + hr) * AVW
                        nc.vector.scalar_tensor_tensor(
                            vk1[:, base + 64: base + 128],
                            psq[:, 128 + hr * 64: 128 + (hr + 1) * 64],
                            agq[:, NTT + tt:NTT + tt + 1],
                            bias_sb[:, 128 + hr * 64: 128 + (hr + 1) * 64],
                            OP.mult, OP.add)
                        nc.vector.scalar_tensor_tensor(
                            vk1[:, base: base + 64],
                            psq[:, 256 + hr * 64: 256 + (hr + 1) * 64],
                            agq[:, 2 * NTT + tt:2 * NTT + tt + 1],
                            bias_sb[:, 256 + hr * 64: 256 + (hr + 1) * 64],
                            OP.mult, OP.add)

                    pstq = psd.tile([128, 128], BF16, tag="pstq")
                    nc.tensor.transpose(pstq[:],
                                        q_nat[:, tt * FPC:(tt + 1) * FPC],
                                        ident[:])
                    nc.vector.tensor_copy(q_T[:, tt * 128:(tt + 1) * 128],
                                          pstq[:])
                    for hr in range(2):
                        base = (tt * 2 + hr) * AVW
                        pstk = psd.tile([64, 128], BF16, tag="pstk")
                        nc.tensor.transpose(pstk[:],
                                            vk1[:, base + 64: base + 128],
                                            ident[:])
                        nc.vector.tensor_copy(
                            k_T[hr * 64:(hr + 1) * 64, tt * 128:(tt + 1) * 128],
                            pstk[:])

            if dbg:
                nc.sync.dma_start(dbg_gb.ap(), gb[:])
                nc.sync.dma_start(dbg_qnat.ap(), q_nat[:, 0:NTT * FPC])
                nc.sync.dma_start(dbg_qT.ap(), q_T[:])
                nc.sync.dma_start(dbg_kT.ap(), k_T[:])
                nc.sync.dma_start(dbg_vk1.ap(), vk1[:])

            # ================ phase E: attention =============================
            with (
                tc.tile_pool(name="epool", bufs=3) as epool,
                tc.tile_pool(name="sm", bufs=2) as sm,
                tc.tile_pool(name="scps", bufs=1, space="PSUM") as scps,
                tc.tile_pool(name="avps", bufs=1, space="PSUM") as avps,
            ):
                for bh in range(4):
                    b, hr = bh // 2, bh % 2
                    tb = b * T
                    av_ps = avps.tile([128, 3072], F32, tag="av_ps")
                    for kt in range(16):
                        E = epool.tile([128, 2048], BF16, tag="E")
                        for half in range(2):
                            sc = scps.tile([128, 1024], F32, tag="sc")
                            for qq in range(2):
                                qc = half * 2 + qq
                                nc.tensor.matmul(
                                    sc[:, qq * 512:(qq + 1) * 512],
                                    k_T[hr * 64:(hr + 1) * 64,
                                        tb + kt * 128: tb + (kt + 1) * 128],
                                    q_T[hr * 64:(hr + 1) * 64,
                                        tb + qc * 512: tb + (qc + 1) * 512],
                                    start=True, stop=True)
                            nc.scalar.activation(
                                E[:, half * 1024:(half + 1) * 1024], sc[:],
                                AF.Exp, scale=SCALE)
                        rhs = vk1[:, ((b * 16 + kt) * 2 + hr) * AVW:
                                  ((b * 16 + kt) * 2 + hr) * AVW + 129]
                        for qt in range(16):
                            nc.tensor.matmul(
                                av_ps[:, _slot(qt):_slot(qt) + 129],
                                E[:, qt * 128:(qt + 1) * 128], rhs,
                                start=(kt == 0 and qt % 3 == 0),
                                stop=(kt == 15), skip_group_check=True)

                    # epilogue: normalize av, collect D and q.(E@K)
                    for bk in range(6):
                        ns = min(3, 16 - bk * 3)
                        src = av_ps[:, bk * 512: bk * 512 + ns * AVW].rearrange(
                            "p (s c) -> p s c", c=AVW)
                        nc.vector.tensor_copy(
                            dvec_all[:, bh * 16 + bk * 3: bh * 16 + bk * 3 + ns],
                            src[:, :, 128:129].squeeze(-1))
                    rr = sm.tile([128, 16], F32, tag="rr")
                    nc.vector.reciprocal(rr[:],
                                         dvec_all[:, bh * 16:(bh + 1) * 16])
                    nc.vector.tensor_copy(
                        rexp[:].rearrange("p (s c) -> p s c", c=DH),
                        rr[:].unsqueeze(-1).to_broadcast([128, 16, DH]))
                    for bk in range(6):
                        ns = min(3, 16 - bk * 3)
                        src = av_ps[:, bk * 512: bk * 512 + ns * AVW].rearrange(
                            "p (s c) -> p s c", c=AVW)
                        nc.vector.tensor_tensor(
                            av_sb[:, bh * 1024 + bk * 3 * DH:
                                  bh * 1024 + (bk * 3 + ns) * DH].rearrange(
                                      "p (s c) -> p s c", c=DH),
                            src[:, :, 0:64],
                            rexp[:, bk * 3 * DH: (bk * 3 + ns) * DH].rearrange(
                                "p (s c) -> p s c", c=DH),
                            OP.mult)
                        qsrc = q_nat[:, (b * 16 + bk * 3) * FPC + hr * 64:
                                     (b * 16 + bk * 3 + ns) * FPC + hr * 64
                                     ].rearrange("p (s c) -> p s c",
                                                 c=FPC)[:, :, 0:64]
                        nc.vector.tensor_tensor(
                            ent_scr[:, bk * 3 * DH:(bk * 3 + ns) * DH].rearrange(
                                "p (s c) -> p s c", c=DH),
                            src[:, :, 64:128], qsrc, OP.mult)
                    nc.vector.tensor_reduce(
                        edot_all[:, bh * 16:(bh + 1) * 16],
                        ent_scr[:].rearrange("p (s c) -> p s c", c=DH),
                        AX.X, OP.add)

                    # av slice of this (b,h) -> a2a_in
                    nc.sync.dma_start(
                        a2a_in.ap().rearrange("j (ql p) f -> p j ql f", p=128)
                        [:, b * 4:(b + 1) * 4, :, hr * 64:(hr + 1) * 64],
                        av_sb[:, bh * 1024:(bh + 1) * 1024].rearrange(
                            "p (qh ql d) -> p qh ql d", qh=4, ql=4))

                nc.gpsimd.collective_compute("AllToAll", OP.bypass,
                                             replica_groups=rg,
                                             ins=[a2a_in.ap()],
                                             outs=[a2a_out.ap()])

            if dbg:
                nc.sync.dma_start(dbg_dvec.ap(), dvec_all[:])
                nc.sync.dma_start(dbg_edot.ap(), edot_all[:])

            # ============ entropy finalize + phase F: O projection ===========
            with (
                tc.tile_pool(name="sm2", bufs=2) as sm2,
                tc.tile_pool(name="psf", bufs=2, space="PSUM") as psf,
            ):
                lnd = sm2.tile([128, 64], F32, tag="lnd")
                nc.scalar.activation(lnd[:], dvec_all[:], AF.Ln)
                rall = sm2.tile([128, 64], F32, tag="rall")
                nc.vector.reciprocal(rall[:], dvec_all[:])
                t2 = sm2.tile([128, 64], F32, tag="t2")
                nc.vector.tensor_tensor(t2[:], edot_all[:], rall[:], OP.mult)
                entbuf = sm2.tile([128, 64], F32, tag="entbuf")
                nc.vector.scalar_tensor_tensor(entbuf[:], t2[:], -SCALE, lnd[:],
                                               OP.mult, OP.add)
                entrow = sm2.tile([128, 1], F32, tag="entrow")
                nc.vector.tensor_reduce(entrow[:], entbuf[:].unsqueeze(1),
                                        AX.X, OP.add)
                pse = psf.tile([1, 1], F32, tag="pse")
                nc.tensor.matmul(pse[:], ones_c[:], entrow[:],
                                 start=True, stop=True)
                ent_sb = sm2.tile([1, 1], F32, tag="ent_sb")
                nc.vector.tensor_copy(ent_sb[:], pse[:])
                nc.sync.dma_start(ent_d.ap(), ent_sb[:])

                for m in range(TOK // 128):
                    avm = sm2.tile([128, D], F32, tag="avm")
                    nc.sync.dma_start(
                        avm[:].rearrange("p (j f) -> p j f", j=8),
                        a2a_out.ap().rearrange("j (mm p) f -> mm p j f",
                                               p=128)[m])
                    rmax = sm2.tile([128, 1], F32, tag="rmaxo")
                    nc.vector.tensor_reduce(rmax[:], avm[:], AX.X, OP.max,
                                            apply_absolute_value=True)
                    rmax2 = sm2.tile([128, 1], F32, tag="rmaxo2")
                    nc.vector.tensor_scalar(rmax2[:], rmax[:], 1e-5, None,
                                            OP.max)
                    srec = sm2.tile([128, 1], F32, tag="sreco")
                    nc.vector.reciprocal(srec[:], rmax2[:])
                    sq = sm2.tile([128, 1], F32, tag="sqo")
                    nc.vector.tensor_scalar(sq[:], srec[:], 127.0, None,
                                            OP.mult)
                    alo = sm2.tile([128, 1], F32, tag="alo")
                    nc.vector.tensor_scalar(alo[:], rmax2[:], gb[:, 3:4],
                                            INV127, OP.mult, OP.mult)
                    t1o = sm2.tile([128, D], F32, tag="t1o")
                    nc.vector.tensor_scalar(t1o[:], avm[:], sq[:], MAGIC,
                                            OP.mult, OP.add)
                    oq = sm2.tile([128, D], BF16, tag="oq")
                    nc.vector.tensor_scalar(oq[:], t1o[:], MAGIC, None,
                                            OP.subtract)

                    if dbg:
                        nc.sync.dma_start(dbg_avm.ap()[m], avm[:])
                        nc.sync.dma_start(dbg_oq.ap()[m], oq[:])
                        nc.sync.dma_start(dbg_alo.ap()[m], alo[:])
                    oqt = sm2.tile([128, D], BF16, tag="oqt")
                    for ib in range(8):
                        pst = psf.tile([128, 128], BF16, tag="psto")
                        nc.tensor.transpose(pst[:],
                                            oq[:, ib * 128:(ib + 1) * 128],
                                            ident[:])
                        nc.vector.tensor_copy(oqt[:, ib * 128:(ib + 1) * 128],
                                              pst[:])

                    fin = sm2.tile([128, D], F32, tag="fin")
                    for of in range(2):
                        pso = psf.tile([128, 512], F32, tag="pso")
                        for ic in range(8):
                            nc.tensor.matmul(
                                pso[:], oqt[:, ic * 128:(ic + 1) * 128],
                                woc_sb[:, ic * D + of * 512:
                                       ic * D + (of + 1) * 512],
                                start=(ic == 0), stop=(ic == 7))
                        nc.vector.scalar_tensor_tensor(
                            fin[:, of * 512:(of + 1) * 512], pso[:], alo[:],
                            bo_sb[:, of * 512:(of + 1) * 512], OP.mult, OP.add)
                    nc.sync.dma_start(out_d.ap()[m * 128:(m + 1) * 128, :],
                                      fin[:])

    nc.compile()
    return nc


_CACHE = {}


def _get_program():
    if "nc" not in _CACHE:
        _CACHE["nc"] = build_program()
    return _CACHE["nc"]


def kernel(x, wq, bq, wk, bk, wv, bv, wo, bo):
    x = np.ascontiguousarray(np.asarray(x, dtype=np.float32))
    wq, wk, wv, wo = (np.asarray(a, dtype=np.float32) for a in (wq, wk, wv, wo))
    bq, bk, bv, bo = (np.asarray(a, dtype=np.float32) for a in (bq, bk, bv, bo))

    xf = x.reshape(N, D)
    bo_b = np.ascontiguousarray(np.broadcast_to(bo[None, :], (128, D)))
    wo_tr = np.ascontiguousarray(wo.T)

    in_maps = []
    for c in range(NCORES):
        rows = slice(c * FPC, (c + 1) * FPC)
        wt = np.concatenate([wq[rows].T, wk[rows].T, wv[rows].T], axis=1)
        bias = np.concatenate([bq[rows], bk[rows], bv[rows]])
        in_maps.append({
            "xs": np.ascontiguousarray(xf[c * TOK:(c + 1) * TOK]),
            "wt_qkv": np.ascontiguousarray(wt),
            "wo_t": wo_tr,
            "bias_qkv": np.ascontiguousarray(
                np.broadcast_to(bias[None, :], (128, 3 * FPC))),
            "bo_bc": bo_b,
        })

    nc = _get_program()
    res = run_bass_kernel_spmd(nc, in_maps, core_ids=list(range(NCORES)),
                               trace=bool(os.environ.get("BITATTN_TRACE")))
    _CACHE["exec_time_ns"] = res.exec_time_ns

    out = np.concatenate([res.results[c]["out"] for c in range(NCORES)], axis=0)
    ent = sum(float(res.results[c]["entp"][0, 0]) for c in range(NCORES))
    entropy = np.float32(ent / (B * H * T))
    return out.reshape(B, T, D), entropy
